# revision 18
# baseline (speedup 1.0000x reference)
"""Bass/Trainium2 kernel for nn_ConvLayer (sparse-GP conv layer conditional).

Computes, for X:[64,1600], Z:[384,25], q_mu:[384,1], q_sqrt:[384,384]:
    patches = im2col(X)                       [N,P,L]   P=1296, L=25
    Kuf = rbf(Z, patches)                     [M, N*P]
    Kuu = rbf(Z,Z) + jitter*I
    A = Kuu^-1 Kuf
    mean = A^T q_mu
    diag = colwise  A^T (Lq Lq^T - Kuu) A
    var  = variance + diag
Returns (mean [N,P], var [N,P]).

Strategy: data-parallel over batch N across 8 NeuronCores (8 images/core).
Host precomputes the shared M x M quantities in float64 (W = Kuu^-1,
Tt = var^2 * W (Lq Lq^T - Kuu) W, wt = var * W q_mu) and replicates them.
Per patch column c the device computes
    sq_c  = Zaug2^T Paug2   (augmented matmul = ||z||^2+||p||^2-2 z.p)
    E_c   = exp(s * sq_c)                      (s = -0.5/ls^2)
    mean  = wt^T E
    diag  = 1^T (E o (Tt E))                   (o = elementwise)
and the host adds `variance` to the diag row.

The per-tile dependency chain sq->exp->mean/H'->prod->reduce is software-
pipelined across three column tiles so the in-order TensorE never waits on
ScalarE/VectorE results of the same tile: iteration i issues
    PE : sq_i | mean_{i-1}, H'_{i-1} | reduce_{i-2}
    ACT: exp_i            DVE: prod_{i-1}        DMA: out_{i-2}
which keeps TensorE continuously busy (full p-state) at its algebraic
floor of 15 matmul-slots per 512-column tile.
"""

import numpy as np
from contextlib import ExitStack
from numpy.lib.stride_tricks import sliding_window_view

import concourse.bass as bass
import concourse.mybir as mybir
import concourse.tile as tile
from concourse import bacc
from concourse.bass_utils import run_bass_kernel_spmd

# Problem constants (hardcoded per spec)
H = 40
WID = 40
KS = 5
HOUT = H - KS + 1            # 36
WOUT = WID - KS + 1          # 36
P = HOUT * WOUT              # 1296
L = KS * KS                  # 25
M = 384                      # inducing points
N = 64                       # batch
JITTER = 1e-6
NCORES = 8
NPC = N // NCORES            # images per core = 8
COLS = NPC * P               # patch columns per core = 10368
LA = L + 2                   # augmented contraction: patches, ||p||^2, ones
LP = 128                     # contraction padded to 128 (27-row matmuls
                             # stream at half rate on TRN2; zero weight rows
                             # make the pad contribution exactly 0)

F32 = mybir.dt.float32
F32R = mybir.dt.float32r
MCH = M // 128               # 3 chunks of the M dim

# column tiles: keep every matmul free-dim >= 256 so float32r runs at
# full rate (19*512 + 2*320 = 10368)
TILES = [(i * 512, 512) for i in range(19)] + [(19 * 512, 320), (19 * 512 + 320, 320)]
NT = len(TILES)


def _build_program(s_scale: float):
    """Build the SPMD single-core Bass program (same on all 8 cores)."""
    nc = bacc.Bacc("TRN2", target_bir_lowering=False, debug=False, num_devices=NCORES)

    d_paug = nc.dram_tensor("paug", [LP, COLS], F32R, kind="ExternalInput").ap()
    d_zaug = nc.dram_tensor("zaugt", [LP, M], F32R, kind="ExternalInput").ap()
    d_rt = nc.dram_tensor("rt", [128, MCH, M], F32R, kind="ExternalInput").ap()
    d_vecr = nc.dram_tensor("vecr", [128, 8], F32R, kind="ExternalInput").ap()
    d_out = nc.dram_tensor("outb", [2, COLS], F32, kind="ExternalOutput").ap()

    with tile.TileContext(nc) as tc, ExitStack() as ctx:
        const = ctx.enter_context(tc.tile_pool(name="const", bufs=1))
        pa_pool = ctx.enter_context(tc.tile_pool(name="pa", bufs=4))
        e_pool = ctx.enter_context(tc.tile_pool(name="epool", bufs=3))
        pr_pool = ctx.enter_context(tc.tile_pool(name="prpool", bufs=3))
        ob_pool = ctx.enter_context(tc.tile_pool(name="obpool", bufs=3))
        ps_a = ctx.enter_context(tc.tile_pool(name="psa", bufs=3, space="PSUM"))
        ps_g = ctx.enter_context(tc.tile_pool(name="psg", bufs=3, space="PSUM"))
        ps_mv = ctx.enter_context(tc.tile_pool(name="psmv", bufs=2, space="PSUM"))

        sb_zaug = const.tile([LP, M], F32R)
        nc.sync.dma_start(sb_zaug[:, :], d_zaug)
        sb_vecr = const.tile([128, 8], F32R)
        nc.sync.dma_start(sb_vecr[:, :], d_vecr)
        # rt rides the scalar-engine DMA queue so the paug tile loads on the
        # sync queue aren't stuck behind 590KB of weights at startup
        sb_rt = const.tile([128, MCH, M], F32R)
        for k in range(MCH):
            nc.scalar.dma_start(sb_rt[:, k, :], d_rt[:, k, :])

        # pipeline state per in-flight tile: (e3, sq_banks, pr, pmv, c0, F)
        st = {}

        for i in range(NT + 2):
            # ---- stage A: sq_i (PE) + exp_i (ACT) ----
            if i < NT:
                c0, F = TILES[i]
                sb_pa = pa_pool.tile([LP, F], F32R)
                nc.sync.dma_start(sb_pa[:, :], d_paug[:, c0:c0 + F])
                e3 = e_pool.tile([128, MCH, F], F32R)
                sqs = []
                for k in range(MCH):
                    pa_ps = ps_a.tile([128, F], F32)
                    nc.tensor.matmul(
                        pa_ps[:, :],
                        lhsT=sb_zaug[:, k * 128:(k + 1) * 128],
                        rhs=sb_pa[:, :],
                        start=True, stop=True,
                    )
                    sqs.append(pa_ps)
                    nc.scalar.activation(
                        e3[:, k, :], pa_ps[:, :],
                        mybir.ActivationFunctionType.Exp,
                        scale=float(s_scale),
                    )
                st[i] = {"e3": e3, "c0": c0, "F": F}

            # ---- stage B: mean_{i-1} + H'_{i-1} (PE), prod_{i-1} (DVE) ----
            if 1 <= i <= NT:
                t = st[i - 1]
                e3, F = t["e3"], t["F"]
                pmv = ps_mv.tile([2, F], F32)
                for k in range(MCH):
                    nc.tensor.matmul(
                        pmv[:, :],
                        lhsT=sb_vecr[:, 2 * k:2 * k + 2],
                        rhs=e3[:, k, :],
                        start=(k == 0), stop=False,
                    )
                # Symmetric half-form: H'_j = sum_{k<j} Tt_jk E_k + 0.5 Tt_jj E_j
                # diag = 2 * sum_j 1^T (E_j o H'_j)   (exact, Tt symmetric)
                pr3 = pr_pool.tile([128, MCH, F], F32R)
                for j in range(MCH):
                    g_ps = ps_g.tile([128, F], F32)
                    for k in range(j + 1):
                        nc.tensor.matmul(
                            g_ps[:, :],
                            lhsT=sb_rt[:, k, j * 128:(j + 1) * 128],
                            rhs=e3[:, k, :],
                            start=(k == 0), stop=(k == j),
                        )
                    nc.vector.tensor_mul(pr3[:, j, :], e3[:, j, :], g_ps[:, :])
                t["pmv"] = pmv
                t["pr3"] = pr3

            # ---- stage C: reduce_{i-2} (PE) + out DMA ----
            if i >= 2:
                t = st.pop(i - 2)
                pmv, pr3, F, c0 = t["pmv"], t["pr3"], t["F"], t["c0"]
                for j in range(MCH):
                    nc.tensor.matmul(
                        pmv[:, :],
                        lhsT=sb_vecr[:, 6:8],
                        rhs=pr3[:, j, :],
                        start=False, stop=(j == MCH - 1),
                    )
                ob = ob_pool.tile([2, F], F32)
                nc.scalar.copy(ob[:, :], pmv[:, :])
                nc.scalar.dma_start(d_out[:, c0:c0 + F], ob[:, :])

    nc.compile()
    return nc


def _host_prep(X, Z, q_mu, q_sqrt, variance, lengthscale):
    var = float(np.asarray(variance).reshape(-1)[0])
    ls = float(np.asarray(lengthscale).reshape(-1)[0])
    s = -0.5 / (ls * ls)

    Z64 = np.asarray(Z, np.float64)
    zz = (Z64 * Z64).sum(1)                                   # [M]
    sq = zz[:, None] + zz[None, :] - 2.0 * (Z64 @ Z64.T)
    np.maximum(sq, 0.0, out=sq)
    Kuu = var * np.exp(s * sq) + JITTER * np.eye(M)
    Wi = np.linalg.inv(Kuu)
    Lq = np.tril(np.asarray(q_sqrt, np.float64))
    SK = Lq @ Lq.T - Kuu
    Tt = (var * var) * (Wi @ SK @ Wi)                         # symmetric
    wt = var * (Wi @ np.asarray(q_mu, np.float64)[:, 0])      # [M]

    # Block upper-triangle (with halved diagonal blocks) of symmetric Tt:
    # H'_j = sum_{k<=j} C_jk E_k needs lhsT rt[m,k,j] = TU[128k+m, j] where
    # TU keeps blocks k<=j only and halves the diagonal blocks.
    TU = Tt.copy().reshape(MCH, 128, MCH, 128)
    for kb in range(MCH):
        for jb in range(MCH):
            if kb > jb:
                TU[kb, :, jb, :] = 0.0
            elif kb == jb:
                TU[kb, :, jb, :] *= 0.5
    TU = TU.reshape(M, M)
    rt = np.ascontiguousarray(
        TU.reshape(MCH, 128, M).transpose(1, 0, 2)
    ).astype(np.float32)

    # vecr pairs: [wt_k, 0] for k=0..2 at cols 0..5; [0, 2] at cols 6..7
    vecr = np.zeros((128, 8), np.float32)
    for k in range(MCH):
        vecr[:, 2 * k] = wt.reshape(MCH, 128)[k]
    vecr[:, 7] = 2.0

    zaugt = np.zeros((LP, M), np.float32)
    zaugt[:L] = -2.0 * Z64.T
    zaugt[L] = 1.0
    zaugt[L + 1] = zz

    # patches + squared norms + ones, per core
    Ximg = np.asarray(X, np.float32).reshape(N, H, WID)
    pw = sliding_window_view(Ximg, (KS, KS), axis=(1, 2))     # [N,36,36,5,5]
    patches = pw.reshape(N, P, L)
    p2 = (patches.astype(np.float64) ** 2).sum(-1)            # [N,P]
    paug = np.zeros((NCORES, LP, COLS), np.float32)
    for c in range(NCORES):
        blk = patches[c * NPC:(c + 1) * NPC].reshape(COLS, L)
        paug[c, :L] = blk.T
        paug[c, L] = p2[c * NPC:(c + 1) * NPC].reshape(-1)
        paug[c, L + 1] = 1.0

    return s, var, zaugt, rt, vecr, paug


def kernel(X, Z, q_mu, q_sqrt, variance, lengthscale, _trace=False, _trace_kwargs=None):
    s, var, zaugt, rt, vecr, paug = _host_prep(
        X, Z, q_mu, q_sqrt, variance, lengthscale)

    nc = _build_program(s)

    in_maps = [
        {"paug": np.ascontiguousarray(paug[c]),
         "zaugt": zaugt, "rt": rt, "vecr": vecr}
        for c in range(NCORES)
    ]
    res = run_bass_kernel_spmd(
        nc, in_maps, list(range(NCORES)),
        trace=_trace, **(_trace_kwargs or {}),
    )

    mean = np.empty((N, P), np.float32)
    varo = np.empty((N, P), np.float32)
    for c in range(NCORES):
        ob = res.results[c]["outb"]
        mean[c * NPC:(c + 1) * NPC] = ob[0].reshape(NPC, P)
        varo[c * NPC:(c + 1) * NPC] = ob[1].reshape(NPC, P) + np.float32(var)
    if _trace:
        return (mean, varo), res
    return mean, varo


# revision 19
# speedup vs baseline: 1.0039x; 1.0039x over previous
"""Bass/Trainium2 kernel for nn_ConvLayer (sparse-GP conv layer conditional).

Computes, for X:[64,1600], Z:[384,25], q_mu:[384,1], q_sqrt:[384,384]:
    patches = im2col(X)                       [N,P,L]   P=1296, L=25
    Kuf = rbf(Z, patches)                     [M, N*P]
    Kuu = rbf(Z,Z) + jitter*I
    A = Kuu^-1 Kuf
    mean = A^T q_mu
    diag = colwise  A^T (Lq Lq^T - Kuu) A
    var  = variance + diag
Returns (mean [N,P], var [N,P]).

Strategy: data-parallel over batch N across 8 NeuronCores (8 images/core).
Host precomputes the shared M x M quantities in float64 (W = Kuu^-1,
Tt = var^2 * W (Lq Lq^T - Kuu) W, wt = var * W q_mu) and replicates them.
Per patch column c the device computes
    sq_c  = Zaug2^T Paug2   (augmented matmul = ||z||^2+||p||^2-2 z.p)
    E_c   = exp(s * sq_c)                      (s = -0.5/ls^2)
    mean  = wt^T E
    diag  = 1^T (E o (Tt E))                   (o = elementwise)
and the host adds `variance` to the diag row.

The per-tile dependency chain sq->exp->mean/H'->prod->reduce is software-
pipelined across three column tiles so the in-order TensorE never waits on
ScalarE/VectorE results of the same tile: iteration i issues
    PE : sq_i | mean_{i-1}, H'_{i-1} | reduce_{i-2}
    ACT: exp_i            DVE: prod_{i-1}        DMA: out_{i-2}
which keeps TensorE continuously busy (full p-state) at its algebraic
floor of 15 matmul-slots per 512-column tile.
"""

import numpy as np
from contextlib import ExitStack
from numpy.lib.stride_tricks import sliding_window_view

import concourse.bass as bass
import concourse.mybir as mybir
import concourse.tile as tile
from concourse import bacc
from concourse.bass_utils import run_bass_kernel_spmd

# Problem constants (hardcoded per spec)
H = 40
WID = 40
KS = 5
HOUT = H - KS + 1            # 36
WOUT = WID - KS + 1          # 36
P = HOUT * WOUT              # 1296
L = KS * KS                  # 25
M = 384                      # inducing points
N = 64                       # batch
JITTER = 1e-6
NCORES = 8
NPC = N // NCORES            # images per core = 8
COLS = NPC * P               # patch columns per core = 10368
LA = L + 2                   # augmented contraction: patches, ||p||^2, ones
LP = 128                     # contraction padded to 128 (27-row matmuls
                             # stream at half rate on TRN2; zero weight rows
                             # make the pad contribution exactly 0)

F32 = mybir.dt.float32
F32R = mybir.dt.float32r
MCH = M // 128               # 3 chunks of the M dim

# column tiles: keep every matmul free-dim >= 256 so float32r runs at
# full rate (19*512 + 2*320 = 10368)
TILES = [(i * 512, 512) for i in range(19)] + [(19 * 512, 320), (19 * 512 + 320, 320)]
NT = len(TILES)


def _build_program(s_scale: float):
    """Build the SPMD single-core Bass program (same on all 8 cores)."""
    nc = bacc.Bacc("TRN2", target_bir_lowering=False, debug=False, num_devices=NCORES)

    d_paug = nc.dram_tensor("paug", [LP, COLS], F32R, kind="ExternalInput").ap()
    d_zaug = nc.dram_tensor("zaugt", [LP, M], F32R, kind="ExternalInput").ap()
    d_rt = nc.dram_tensor("rt", [128, MCH, M], F32R, kind="ExternalInput").ap()
    d_vecr = nc.dram_tensor("vecr", [128, 8], F32R, kind="ExternalInput").ap()
    d_out = nc.dram_tensor("outb", [2, COLS], F32, kind="ExternalOutput").ap()

    with tile.TileContext(nc) as tc, ExitStack() as ctx:
        const = ctx.enter_context(tc.tile_pool(name="const", bufs=1))
        pa_pool = ctx.enter_context(tc.tile_pool(name="pa", bufs=4))
        e_pool = ctx.enter_context(tc.tile_pool(name="epool", bufs=3))
        pr_pool = ctx.enter_context(tc.tile_pool(name="prpool", bufs=3))
        ob_pool = ctx.enter_context(tc.tile_pool(name="obpool", bufs=3))
        ps_a = ctx.enter_context(tc.tile_pool(name="psa", bufs=3, space="PSUM"))
        ps_g = ctx.enter_context(tc.tile_pool(name="psg", bufs=3, space="PSUM"))
        ps_mv = ctx.enter_context(tc.tile_pool(name="psmv", bufs=2, space="PSUM"))

        sb_zaug = const.tile([LP, M], F32R)
        nc.sync.dma_start(sb_zaug[:, :], d_zaug)
        sb_vecr = const.tile([128, 8], F32R)
        nc.sync.dma_start(sb_vecr[:, :], d_vecr)
        # rt rides the scalar-engine DMA queue so the paug tile loads on the
        # sync queue aren't stuck behind 590KB of weights at startup
        sb_rt = const.tile([128, MCH, M], F32R)
        for k in range(MCH):
            nc.scalar.dma_start(sb_rt[:, k, :], d_rt[:, k, :])

        # pipeline state per in-flight tile: (e3, sq_banks, pr, pmv, c0, F)
        st = {}

        for i in range(NT + 2):
            # ---- stage A: sq_i (PE) + exp_i (ACT) ----
            if i < NT:
                c0, F = TILES[i]
                sb_pa = pa_pool.tile([LP, F], F32R)
                nc.sync.dma_start(sb_pa[:, :], d_paug[:, c0:c0 + F])
                e3 = e_pool.tile([128, MCH, F], F32R)
                sqs = []
                for k in range(MCH):
                    pa_ps = ps_a.tile([128, F], F32)
                    nc.tensor.matmul(
                        pa_ps[:, :],
                        lhsT=sb_zaug[:, k * 128:(k + 1) * 128],
                        rhs=sb_pa[:, :],
                        start=True, stop=True,
                    )
                    sqs.append(pa_ps)
                    nc.scalar.activation(
                        e3[:, k, :], pa_ps[:, :],
                        mybir.ActivationFunctionType.Exp,
                        scale=float(s_scale),
                    )
                st[i] = {"e3": e3, "c0": c0, "F": F}

            # ---- stage B: mean_{i-1} + H'_{i-1} (PE), prod_{i-1} (DVE) ----
            if 1 <= i <= NT:
                t = st[i - 1]
                e3, F = t["e3"], t["F"]
                pmv = ps_mv.tile([2, F], F32)
                for k in range(MCH):
                    nc.tensor.matmul(
                        pmv[:, :],
                        lhsT=sb_vecr[:, 2 * k:2 * k + 2],
                        rhs=e3[:, k, :],
                        start=(k == 0), stop=False,
                    )
                # Symmetric half-form: H'_j = sum_{k<j} Tt_jk E_k + 0.5 Tt_jj E_j
                # diag = 2 * sum_j 1^T (E_j o H'_j)   (exact, Tt symmetric)
                pr3 = pr_pool.tile([128, MCH, F], F32R)
                for j in range(MCH):
                    g_ps = ps_g.tile([128, F], F32)
                    for k in range(j + 1):
                        nc.tensor.matmul(
                            g_ps[:, :],
                            lhsT=sb_rt[:, k, j * 128:(j + 1) * 128],
                            rhs=e3[:, k, :],
                            start=(k == 0), stop=(k == j),
                        )
                    nc.vector.tensor_mul(pr3[:, j, :], e3[:, j, :], g_ps[:, :])
                t["pmv"] = pmv
                t["pr3"] = pr3

            # ---- stage C: reduce_{i-2} (PE) + out DMA ----
            if i >= 2:
                t = st.pop(i - 2)
                pmv, pr3, F, c0 = t["pmv"], t["pr3"], t["F"], t["c0"]
                for j in range(MCH):
                    nc.tensor.matmul(
                        pmv[:, :],
                        lhsT=sb_vecr[:, 6:8],
                        rhs=pr3[:, j, :],
                        start=False, stop=(j == MCH - 1),
                    )
                ob = ob_pool.tile([2, F], F32)
                nc.scalar.copy(ob[:, :], pmv[:, :])
                nc.sync.dma_start(d_out[:, c0:c0 + F], ob[:, :])

    nc.compile()
    return nc


def _host_prep(X, Z, q_mu, q_sqrt, variance, lengthscale):
    var = float(np.asarray(variance).reshape(-1)[0])
    ls = float(np.asarray(lengthscale).reshape(-1)[0])
    s = -0.5 / (ls * ls)

    Z64 = np.asarray(Z, np.float64)
    zz = (Z64 * Z64).sum(1)                                   # [M]
    sq = zz[:, None] + zz[None, :] - 2.0 * (Z64 @ Z64.T)
    np.maximum(sq, 0.0, out=sq)
    Kuu = var * np.exp(s * sq) + JITTER * np.eye(M)
    Wi = np.linalg.inv(Kuu)
    Lq = np.tril(np.asarray(q_sqrt, np.float64))
    SK = Lq @ Lq.T - Kuu
    Tt = (var * var) * (Wi @ SK @ Wi)                         # symmetric
    wt = var * (Wi @ np.asarray(q_mu, np.float64)[:, 0])      # [M]

    # Block upper-triangle (with halved diagonal blocks) of symmetric Tt:
    # H'_j = sum_{k<=j} C_jk E_k needs lhsT rt[m,k,j] = TU[128k+m, j] where
    # TU keeps blocks k<=j only and halves the diagonal blocks.
    TU = Tt.copy().reshape(MCH, 128, MCH, 128)
    for kb in range(MCH):
        for jb in range(MCH):
            if kb > jb:
                TU[kb, :, jb, :] = 0.0
            elif kb == jb:
                TU[kb, :, jb, :] *= 0.5
    TU = TU.reshape(M, M)
    rt = np.ascontiguousarray(
        TU.reshape(MCH, 128, M).transpose(1, 0, 2)
    ).astype(np.float32)

    # vecr pairs: [wt_k, 0] for k=0..2 at cols 0..5; [0, 2] at cols 6..7
    vecr = np.zeros((128, 8), np.float32)
    for k in range(MCH):
        vecr[:, 2 * k] = wt.reshape(MCH, 128)[k]
    vecr[:, 7] = 2.0

    zaugt = np.zeros((LP, M), np.float32)
    zaugt[:L] = -2.0 * Z64.T
    zaugt[L] = 1.0
    zaugt[L + 1] = zz

    # patches + squared norms + ones, per core
    Ximg = np.asarray(X, np.float32).reshape(N, H, WID)
    pw = sliding_window_view(Ximg, (KS, KS), axis=(1, 2))     # [N,36,36,5,5]
    patches = pw.reshape(N, P, L)
    p2 = (patches.astype(np.float64) ** 2).sum(-1)            # [N,P]
    paug = np.zeros((NCORES, LP, COLS), np.float32)
    for c in range(NCORES):
        blk = patches[c * NPC:(c + 1) * NPC].reshape(COLS, L)
        paug[c, :L] = blk.T
        paug[c, L] = p2[c * NPC:(c + 1) * NPC].reshape(-1)
        paug[c, L + 1] = 1.0

    return s, var, zaugt, rt, vecr, paug


def kernel(X, Z, q_mu, q_sqrt, variance, lengthscale, _trace=False, _trace_kwargs=None):
    s, var, zaugt, rt, vecr, paug = _host_prep(
        X, Z, q_mu, q_sqrt, variance, lengthscale)

    nc = _build_program(s)

    in_maps = [
        {"paug": np.ascontiguousarray(paug[c]),
         "zaugt": zaugt, "rt": rt, "vecr": vecr}
        for c in range(NCORES)
    ]
    res = run_bass_kernel_spmd(
        nc, in_maps, list(range(NCORES)),
        trace=_trace, **(_trace_kwargs or {}),
    )

    mean = np.empty((N, P), np.float32)
    varo = np.empty((N, P), np.float32)
    for c in range(NCORES):
        ob = res.results[c]["outb"]
        mean[c * NPC:(c + 1) * NPC] = ob[0].reshape(NPC, P)
        varo[c * NPC:(c + 1) * NPC] = ob[1].reshape(NPC, P) + np.float32(var)
    if _trace:
        return (mean, varo), res
    return mean, varo


# revision 21
# speedup vs baseline: 1.0122x; 1.0083x over previous
"""Bass/Trainium2 kernel for nn_ConvLayer (sparse-GP conv layer conditional).

Computes, for X:[64,1600], Z:[384,25], q_mu:[384,1], q_sqrt:[384,384]:
    patches = im2col(X)                       [N,P,L]   P=1296, L=25
    Kuf = rbf(Z, patches)                     [M, N*P]
    Kuu = rbf(Z,Z) + jitter*I
    A = Kuu^-1 Kuf
    mean = A^T q_mu
    diag = colwise  A^T (Lq Lq^T - Kuu) A
    var  = variance + diag
Returns (mean [N,P], var [N,P]).

Strategy: data-parallel over batch N across 8 NeuronCores (8 images/core).
Host precomputes the shared M x M quantities in float64 (W = Kuu^-1,
Tt = var^2 * W (Lq Lq^T - Kuu) W, wt = var * W q_mu) and replicates them.
Per patch column c the device computes
    sq_c  = Zaug2^T Paug2   (augmented matmul = ||z||^2+||p||^2-2 z.p)
    E_c   = exp(s * sq_c)                      (s = -0.5/ls^2)
    mean  = wt^T E
    diag  = 1^T (E o (Tt E))                   (o = elementwise)
and the host adds `variance` to the diag row.

The per-tile dependency chain sq->exp->mean/H'->prod->reduce is software-
pipelined across three column tiles so the in-order TensorE never waits on
ScalarE/VectorE results of the same tile: iteration i issues
    PE : sq_i | mean_{i-1}, H'_{i-1} | reduce_{i-2}
    ACT: exp_i            DVE: prod_{i-1}        DMA: out_{i-2}
which keeps TensorE continuously busy (full p-state) at its algebraic
floor of 15 matmul-slots per 512-column tile.
"""

import numpy as np
from contextlib import ExitStack
from numpy.lib.stride_tricks import sliding_window_view

import concourse.bass as bass
import concourse.mybir as mybir
import concourse.tile as tile
from concourse import bacc
from concourse.bass_utils import run_bass_kernel_spmd

# Problem constants (hardcoded per spec)
H = 40
WID = 40
KS = 5
HOUT = H - KS + 1            # 36
WOUT = WID - KS + 1          # 36
P = HOUT * WOUT              # 1296
L = KS * KS                  # 25
M = 384                      # inducing points
N = 64                       # batch
JITTER = 1e-6
NCORES = 8
NPC = N // NCORES            # images per core = 8
COLS = NPC * P               # patch columns per core = 10368
LA = L + 2                   # augmented contraction: patches, ||p||^2, ones
LP = 128                     # contraction padded to 128 (27-row matmuls
                             # stream at half rate on TRN2; zero weight rows
                             # make the pad contribution exactly 0)

F32 = mybir.dt.float32
F32R = mybir.dt.float32r
MCH = M // 128               # 3 chunks of the M dim

# column tiles: keep every matmul free-dim >= 256 so float32r runs at
# full rate (19*512 + 2*320 = 10368)
TILES = [(i * 512, 512) for i in range(19)] + [(19 * 512, 320), (19 * 512 + 320, 320)]
NT = len(TILES)


def _build_program(s_scale: float):
    """Build the SPMD single-core Bass program (same on all 8 cores)."""
    nc = bacc.Bacc("TRN2", target_bir_lowering=False, debug=False, num_devices=NCORES)

    d_paug = nc.dram_tensor("paug", [LP, COLS], F32R, kind="ExternalInput").ap()
    d_zaug = nc.dram_tensor("zaugt", [LP, M], F32R, kind="ExternalInput").ap()
    d_rt = nc.dram_tensor("rt", [128, MCH, M], F32R, kind="ExternalInput").ap()
    d_vecr = nc.dram_tensor("vecr", [128, 8], F32R, kind="ExternalInput").ap()
    d_out = nc.dram_tensor("outb", [2, COLS], F32, kind="ExternalOutput").ap()

    with tile.TileContext(nc) as tc, ExitStack() as ctx:
        const = ctx.enter_context(tc.tile_pool(name="const", bufs=1))
        pa_pool = ctx.enter_context(tc.tile_pool(name="pa", bufs=4))
        e_pool = ctx.enter_context(tc.tile_pool(name="epool", bufs=3))
        pr_pool = ctx.enter_context(tc.tile_pool(name="prpool", bufs=3))
        ob_pool = ctx.enter_context(tc.tile_pool(name="obpool", bufs=3))
        ps_a = ctx.enter_context(tc.tile_pool(name="psa", bufs=3, space="PSUM"))
        ps_g = ctx.enter_context(tc.tile_pool(name="psg", bufs=3, space="PSUM"))
        ps_mv = ctx.enter_context(tc.tile_pool(name="psmv", bufs=2, space="PSUM"))

        sb_zaug = const.tile([LP, M], F32R)
        nc.sync.dma_start(sb_zaug[:, :], d_zaug)
        sb_vecr = const.tile([128, 8], F32R)
        nc.sync.dma_start(sb_vecr[:, :], d_vecr)
        # rt is DMA'd after the first paug tile (emitted inside iteration 0)
        # so the first sq matmuls aren't stuck behind 590KB of weights
        sb_rt = const.tile([128, MCH, M], F32R)

        # pipeline state per in-flight tile: (e3, sq_banks, pr, pmv, c0, F)
        st = {}

        for i in range(NT + 2):
            # ---- stage A: sq_i (PE) + exp_i (ACT) ----
            if i < NT:
                c0, F = TILES[i]
                sb_pa = pa_pool.tile([LP, F], F32R)
                nc.sync.dma_start(sb_pa[:, :], d_paug[:, c0:c0 + F])
                if i == 0:
                    for k in range(MCH):
                        nc.sync.dma_start(sb_rt[:, k, :], d_rt[:, k, :])
                e3 = e_pool.tile([128, MCH, F], F32R)
                sqs = []
                for k in range(MCH):
                    pa_ps = ps_a.tile([128, F], F32)
                    nc.tensor.matmul(
                        pa_ps[:, :],
                        lhsT=sb_zaug[:, k * 128:(k + 1) * 128],
                        rhs=sb_pa[:, :],
                        start=True, stop=True,
                    )
                    sqs.append(pa_ps)
                    nc.scalar.activation(
                        e3[:, k, :], pa_ps[:, :],
                        mybir.ActivationFunctionType.Exp,
                        scale=float(s_scale),
                    )
                st[i] = {"e3": e3, "c0": c0, "F": F}

            # ---- stage B: mean_{i-1} + H'_{i-1} (PE), prod_{i-1} (DVE) ----
            if 1 <= i <= NT:
                t = st[i - 1]
                e3, F = t["e3"], t["F"]
                pmv = ps_mv.tile([2, F], F32)
                for k in range(MCH):
                    nc.tensor.matmul(
                        pmv[:, :],
                        lhsT=sb_vecr[:, 2 * k:2 * k + 2],
                        rhs=e3[:, k, :],
                        start=(k == 0), stop=False,
                    )
                # Symmetric half-form: H'_j = sum_{k<j} Tt_jk E_k + 0.5 Tt_jj E_j
                # diag = 2 * sum_j 1^T (E_j o H'_j)   (exact, Tt symmetric)
                pr3 = pr_pool.tile([128, MCH, F], F32R)
                for j in range(MCH):
                    g_ps = ps_g.tile([128, F], F32)
                    for k in range(j + 1):
                        nc.tensor.matmul(
                            g_ps[:, :],
                            lhsT=sb_rt[:, k, j * 128:(j + 1) * 128],
                            rhs=e3[:, k, :],
                            start=(k == 0), stop=(k == j),
                        )
                    nc.vector.tensor_mul(pr3[:, j, :], e3[:, j, :], g_ps[:, :])
                t["pmv"] = pmv
                t["pr3"] = pr3

            # ---- stage C: reduce_{i-2} (PE) + out DMA ----
            if i >= 2:
                t = st.pop(i - 2)
                pmv, pr3, F, c0 = t["pmv"], t["pr3"], t["F"], t["c0"]
                for j in range(MCH):
                    nc.tensor.matmul(
                        pmv[:, :],
                        lhsT=sb_vecr[:, 6:8],
                        rhs=pr3[:, j, :],
                        start=False, stop=(j == MCH - 1),
                    )
                ob = ob_pool.tile([2, F], F32)
                nc.scalar.copy(ob[:, :], pmv[:, :])
                nc.sync.dma_start(d_out[:, c0:c0 + F], ob[:, :])

    nc.compile()
    return nc


def _host_prep(X, Z, q_mu, q_sqrt, variance, lengthscale):
    var = float(np.asarray(variance).reshape(-1)[0])
    ls = float(np.asarray(lengthscale).reshape(-1)[0])
    s = -0.5 / (ls * ls)

    Z64 = np.asarray(Z, np.float64)
    zz = (Z64 * Z64).sum(1)                                   # [M]
    sq = zz[:, None] + zz[None, :] - 2.0 * (Z64 @ Z64.T)
    np.maximum(sq, 0.0, out=sq)
    Kuu = var * np.exp(s * sq) + JITTER * np.eye(M)
    Wi = np.linalg.inv(Kuu)
    Lq = np.tril(np.asarray(q_sqrt, np.float64))
    SK = Lq @ Lq.T - Kuu
    Tt = (var * var) * (Wi @ SK @ Wi)                         # symmetric
    wt = var * (Wi @ np.asarray(q_mu, np.float64)[:, 0])      # [M]

    # Block upper-triangle (with halved diagonal blocks) of symmetric Tt:
    # H'_j = sum_{k<=j} C_jk E_k needs lhsT rt[m,k,j] = TU[128k+m, j] where
    # TU keeps blocks k<=j only and halves the diagonal blocks.
    TU = Tt.copy().reshape(MCH, 128, MCH, 128)
    for kb in range(MCH):
        for jb in range(MCH):
            if kb > jb:
                TU[kb, :, jb, :] = 0.0
            elif kb == jb:
                TU[kb, :, jb, :] *= 0.5
    TU = TU.reshape(M, M)
    rt = np.ascontiguousarray(
        TU.reshape(MCH, 128, M).transpose(1, 0, 2)
    ).astype(np.float32)

    # vecr pairs: [wt_k, 0] for k=0..2 at cols 0..5; [0, 2] at cols 6..7
    vecr = np.zeros((128, 8), np.float32)
    for k in range(MCH):
        vecr[:, 2 * k] = wt.reshape(MCH, 128)[k]
    vecr[:, 7] = 2.0

    zaugt = np.zeros((LP, M), np.float32)
    zaugt[:L] = -2.0 * Z64.T
    zaugt[L] = 1.0
    zaugt[L + 1] = zz

    # patches + squared norms + ones, per core
    Ximg = np.asarray(X, np.float32).reshape(N, H, WID)
    pw = sliding_window_view(Ximg, (KS, KS), axis=(1, 2))     # [N,36,36,5,5]
    patches = pw.reshape(N, P, L)
    p2 = (patches.astype(np.float64) ** 2).sum(-1)            # [N,P]
    paug = np.zeros((NCORES, LP, COLS), np.float32)
    for c in range(NCORES):
        blk = patches[c * NPC:(c + 1) * NPC].reshape(COLS, L)
        paug[c, :L] = blk.T
        paug[c, L] = p2[c * NPC:(c + 1) * NPC].reshape(-1)
        paug[c, L + 1] = 1.0

    return s, var, zaugt, rt, vecr, paug


def kernel(X, Z, q_mu, q_sqrt, variance, lengthscale, _trace=False, _trace_kwargs=None):
    s, var, zaugt, rt, vecr, paug = _host_prep(
        X, Z, q_mu, q_sqrt, variance, lengthscale)

    nc = _build_program(s)

    in_maps = [
        {"paug": np.ascontiguousarray(paug[c]),
         "zaugt": zaugt, "rt": rt, "vecr": vecr}
        for c in range(NCORES)
    ]
    res = run_bass_kernel_spmd(
        nc, in_maps, list(range(NCORES)),
        trace=_trace, **(_trace_kwargs or {}),
    )

    mean = np.empty((N, P), np.float32)
    varo = np.empty((N, P), np.float32)
    for c in range(NCORES):
        ob = res.results[c]["outb"]
        mean[c * NPC:(c + 1) * NPC] = ob[0].reshape(NPC, P)
        varo[c * NPC:(c + 1) * NPC] = ob[1].reshape(NPC, P) + np.float32(var)
    if _trace:
        return (mean, varo), res
    return mean, varo
